# revision 30
# baseline (speedup 1.0000x reference)
"""Trainium2 distributed kernel for channel-attention (XCA-style) module.

Reference computation (B=4, C=384, HEADS=8, HD=48, H=W=128, N=HW=16384):
  q = l2norm(in1.view(B,HEADS,HD,N), dim=-1)
  k = l2norm(in2.view(B,HEADS,HD,N), dim=-1)
  attn = softmax(q @ k^T * temperature, dim=-1)          # [B,HEADS,HD,HD]
  out  = attn @ k                                        # [B,HEADS,HD,N]
  out  = proj_w @ out + proj_b                           # 1x1 conv

Distribution: 2D over (batch, spatial-half) — core 2b+h owns batch b and
writes output spatial positions [h*8192, (h+1)*8192). The Gram matrix
q@k^T (contraction over ALL of N) is computed REDUNDANTLY on both cores of
a batch pair from the full fp8 q/k transpose — 6.3 MB of extra DMA per
core buys the removal of the inter-core AllReduce and its latency chain
entirely; there are NO collectives. Softmax + projection-fold run once per
core, the big output matmul covers only the core's spatial half, and the
host concatenates the halves.

Key algebraic tricks:
- Per-head attention + the 1x1-conv projection fuse into ONE matmul:
    final = (proj_w @ blockdiag(attn_h * s_k)) @ k,  s_k[d] = 1/||k_d||.
- The Gram is only needed on the 8 diagonal 48x48 head blocks. Each 128-row
  tile ct only needs head-aligned columns [r0_ct, r0_ct+w_ct) (144/192/144
  of 384), cutting Gram matmul cycles 2.4x. Softmax reads the f32 Gram
  straight out of PSUM; the -1e30 additive mask zeroes in-range off-block
  entries, and the result lands in a persistent pre-zeroed [128, C]
  block-diagonal tile so the fold matmul stays dense.
- q/k row norms and temperature are input statistics; the host precomputes
  the rank-1 logit scale s_q(c)*temp(h)*s_k(d) and the s_k output fold.
- proj_b is applied as a per-partition bias in the PSUM->SBUF output copy
  (scalar/vector engines alternating), not as an extra matmul.
- qkt is host-swizzled so every DMA is a single contiguous 6 KB/partition
  block; phase-D PSUM tiles span two banks so one wide copy retires two
  matmul accumulation groups.
Matmul operands are bf16/fp8 (fp32 accumulation in PSUM); softmax stays f32;
the output is written bf16 and upcast to f32 on the host.
"""

import sys

import numpy as np

try:
    import concourse  # noqa: F401
except ImportError:
    sys.path.insert(0, "/opt/trn_rl_repo")

B, C, HEADS, HD = 4, 384, 8, 48
H = W = 128
N = H * W            # 16384
NCORES = 8
NHALF = 2            # spatial halves per batch
NL = N // NHALF      # 8192 output spatial positions per core
NTF = N // 128       # 128 n-tiles for the (full-N) Gram
NGR = NTF // 8       # 16 swizzled qkt groups
CT = C // 128        # 3 channel tiles
NT4 = NL // 512      # 16 output n-chunks
G4 = 4               # output n-chunks staged per SBUF tile
NEG = -1.0e30
R0 = [0, 96, 240]    # first needed Gram column per channel tile
WR = [144, 192, 144]  # needed Gram column count per channel tile
OFF = [0, 144, 336]  # offsets of the restricted tiles in packed buffers
WTOT = 480
MSCALE = 4096.0      # fp8 range lift for the folded projection matrix


def build_nc(nrep=1):
    import concourse.bass as bass
    import concourse.bacc as bacc
    import concourse.mybir as mybir
    from concourse.tile import TileContext

    f32 = mybir.dt.float32
    bf16 = mybir.dt.bfloat16
    fp8 = mybir.dt.float8e4
    AX = mybir.AxisListType
    AF = mybir.ActivationFunctionType
    DR = mybir.MatmulPerfMode.DoubleRow
    ALU = mybir.AluOpType

    nc = bacc.Bacc()
    nc._allow_low_precision_reason = "bf16/fp8 matmul operands are intentional"

    qkt = nc.declare_dram_parameter("qkt", [NGR, 128, 8, 2 * C], fp8,
                                    isOutput=False)
    # packed constants: one bf16 block (pwt | maskr) and one f32 block
    # (exp-scale | skc | biascol) -> few DMAs; identity for PE transposes
    cb16 = nc.declare_dram_parameter("cb16", [128, CT * C + WTOT], bf16,
                                     isOutput=False)
    cf32 = nc.declare_dram_parameter("cf32", [128, 3 * CT], f32,
                                     isOutput=False)
    ident = nc.declare_dram_parameter("ident", [128, 128], fp8,
                                      isOutput=False)
    out = nc.declare_dram_parameter("out", [C, NL], bf16, isOutput=True)

    with TileContext(nc) as tc:
        with (
            tc.tile_pool(name="const", bufs=1) as cpool,
            tc.tile_pool(name="qk", bufs=4) as qkpool,
            tc.tile_pool(name="work", bufs=8) as wpool,
            tc.tile_pool(name="osb", bufs=6) as opool,
        ):
            # ---- constants (two packed DMAs, issued after the first
            # qkt load of rep 0 so the Gram's data goes out first) ----
            cb16_sb = cpool.tile([128, CT * C + WTOT], bf16)
            cf32_sb = cpool.tile([128, 3 * CT], f32)
            pwt_sb = [cb16_sb[:, ct * C:(ct + 1) * C] for ct in range(CT)]
            maskr_sb = cb16_sb[:, CT * C:CT * C + WTOT]
            expsc_sb = cf32_sb[:, 0:CT]
            skc_sb = cf32_sb[:, CT:2 * CT]
            biascol_sb = cf32_sb[:, 2 * CT:3 * CT]
            # persistent block-diagonal attention tiles; zeros off the
            # restricted ranges are never rewritten
            bd_sb = []
            for ct in range(CT):
                t = cpool.tile([128, C], bf16, name=f"bd{ct}")
                nc.vector.memset(t[:, :], 0.0)
                bd_sb.append(t)
            # persistent fp8 operand tiles for the DoubleRow phase D: dim1
            # indexes the contraction k-tile; slot 3 is a zero pad so the
            # 384-deep contraction runs as two 256-deep DoubleRow pairs
            knp = cpool.tile([128, 4, NL], fp8, name="knp")
            mtp = cpool.tile([128, 4, C], fp8, name="mtp")
            ident_sb = cpool.tile([128, 128], fp8, name="ident")
            nc.gpsimd.memset(knp[:, 3, :], 0.0)
            nc.gpsimd.memset(mtp[:, 3, :], 0.0)

            for rep in range(nrep):
              R = str(rep)

              # ---- phase A: full-N Gram (redundant per batch pair),
              # fp8 DoubleRow: each matmul contracts TWO 128-spatial tiles.
              # The host orders each core's OWN spatial half into groups
              # 8..15; those k-columns are PE-transposed into knp, replacing
              # a separate kn DMA stream entirely ----
              psA_cm = tc.tile_pool(name=f"psA{R}", bufs=1, space="PSUM")
              psA = psA_cm.__enter__()
              psT_cm = tc.tile_pool(name=f"psT{R}", bufs=2, space="PSUM")
              psT = psT_cm.__enter__()
              gram_ps = [psA.tile([128, WR[ct]], f32, name=f"g{R}_{ct}",
                                  tag=f"gram{ct}")
                         for ct in range(CT)]
              ncopy = 0
              for g8 in range(NGR):
                  qk8 = qkpool.tile([128, 8, 2 * C], fp8, name=f"qk{R}_{g8}",
                                    tag="qk", bufs=6)
                  nc.sync.dma_start(qk8[:, :, :], qkt[g8, :, :, :])
                  if rep == 0 and g8 == 0:
                      nc.sync.dma_start(cf32_sb[:, :], cf32[:, :])
                      nc.sync.dma_start(ident_sb[:, :], ident[:, :])
                  if rep == 0 and g8 == 12:
                      # maskr: needed right at softmax; pwt follows the
                      # last qkt group (needed ~2us later at the fold)
                      nc.sync.dma_start(cb16_sb[:, CT * C:],
                                        cb16[:, CT * C:])
                  gm = range(4) if g8 < NGR - 1 else None
                  if gm is not None:
                      for p in gm:
                          for ct in range(CT):
                              nc.tensor.matmul(
                                  gram_ps[ct][:, :],
                                  qk8[:, 2 * p:2 * p + 2,
                                      ct * 128:(ct + 1) * 128],
                                  qk8[:, 2 * p:2 * p + 2,
                                      C + R0[ct]:C + R0[ct] + WR[ct]],
                                  start=(g8 == 0 and p == 0), stop=False,
                                  perf_mode=DR,
                              )
                  else:
                      # final group ct-major: tile ct finishes (stop=True) as
                      # early as possible so its softmax overlaps the rest
                      for ct in range(CT):
                          for p in range(4):
                              nc.tensor.matmul(
                                  gram_ps[ct][:, :],
                                  qk8[:, 2 * p:2 * p + 2,
                                      ct * 128:(ct + 1) * 128],
                                  qk8[:, 2 * p:2 * p + 2,
                                      C + R0[ct]:C + R0[ct] + WR[ct]],
                                  start=False, stop=(p == 3),
                                  perf_mode=DR,
                              )
                  if 1 <= g8 <= 8:
                      # own-half k columns -> knp via PE transpose; fp8
                      # transpose writes PSUM at element step 2, so the
                      # tile is 2x wide and read back strided. Early
                      # positions keep the Act/DVE copies clear of the
                      # softmax and the PE clear of the fold.
                      n0 = (g8 - 1) * 1024
                      for j in range(CT):
                          pt = psT.tile([128, 2 * 8 * 128], fp8,
                                        name=f"t{R}_{g8}_{j}", tag="tp")
                          for ht in range(8):
                              nc.tensor.transpose(
                                  pt[:, ht * 256:(ht + 1) * 256:2],
                                  qk8[:, ht, C + j * 128:C + (j + 1) * 128],
                                  ident_sb[:, :])
                          if ncopy % 2 == 0:
                              nc.scalar.activation(
                                  knp[:, j, n0:n0 + 1024],
                                  pt[:, 0:2048:2], AF.Copy)
                          else:
                              nc.vector.tensor_copy(
                                  knp[:, j, n0:n0 + 1024], pt[:, 0:2048:2])
                          ncopy += 1
              if rep == 0:
                  nc.sync.dma_start(cb16_sb[:, :CT * C], cb16[:, :CT * C])

              # ---- phase C: masked softmax straight from PSUM, fused M^T ----
              # Host ships q/128||q||, k/128||k|| in fp8 (power-of-2 scaling
              # is exact), so logits = gram * temp/16384, folded into the
              # exp's per-partition scale; the -1e30 mask survives the tiny
              # scale (exp(-6e25) == 0).
              for ct in range(CT):
                  w = WR[ct]
                  l = wpool.tile([128, w], f32, name=f"l{R}_{ct}", tag=f"l{ct}")
                  nc.vector.tensor_add(
                      l[:, :], gram_ps[ct][:, :],
                      maskr_sb[:, OFF[ct]:OFF[ct] + w])
                  e = wpool.tile([128, w], f32, name=f"e{R}_{ct}", tag=f"e{ct}")
                  ssum = wpool.tile([128, 1], f32, name=f"ss{R}_{ct}",
                                    tag=f"ss{ct}")
                  # row sum accumulated inside the exp pass (no DVE reduce)
                  nc.scalar.activation(e[:, :], l[:, :], AF.Exp,
                                       scale=expsc_sb[:, ct:ct + 1],
                                       accum_out=ssum[:, :])
                  nc.vector.reciprocal(ssum[:, :], ssum[:, :])
                  # normalized softmax written straight into the persistent
                  # block-diagonal tile (off-range stays zero)
                  nc.vector.tensor_scalar_mul(
                      bd_sb[ct][:, R0[ct]:R0[ct] + w], e[:, :], ssum[:, 0:1])
              psT_cm.__exit__(None, None, None)
              psA_cm.__exit__(None, None, None)

              # ---- fold + phase D share ONE PSUM pool (4 x 2-bank ring):
              # the three fold tiles are the first ring slots, so the first
              # true phase-D tile lands in virgin banks with no pool-release
              # cascade in between ----
              psD_cm = tc.tile_pool(name=f"psD{R}", bufs=4, space="PSUM")
              psD = psD_cm.__enter__()
              # fold matmuls ct-outer: each ct's contribution to all three
              # M^T blocks issues as soon as that ct's softmax lands, so the
              # fold overlaps the remaining softmax columns
              mt_ps = [psD.tile([128, 2 * 512], f32, name=f"mt{R}_{j}",
                                tag="ops") for j in range(CT)]
              for ct in range(CT):
                  for j in range(CT):
                      nc.tensor.matmul(
                          mt_ps[j][:, 0:C],
                          bd_sb[ct][:, j * 128:(j + 1) * 128],
                          pwt_sb[ct],
                          start=(ct == 0), stop=(ct == CT - 1))
              for j in range(CT):
                  # fold MSCALE/128 (the kh stream carries 128*k-hat) into
                  # the PSUM->SBUF fp8 quantization copy
                  if j % 2 == 0:
                      nc.scalar.activation(mtp[:, j, :], mt_ps[j][:, 0:C],
                                           AF.Copy,
                                           scale=skc_sb[:, j:j + 1])
                  else:
                      nc.vector.tensor_scalar_mul(
                          mtp[:, j, :], mt_ps[j][:, 0:C], skc_sb[:, j:j + 1])
              dtiles = []
              for q2 in range(8):
                  for ot in range(CT):
                      if q2 == 7 and ot == 2:
                          dtiles.append((ot, 14, 1))
                          dtiles.append((ot, 15, 1))
                      else:
                          dtiles.append((ot, q2 * 2, 2))
              def d_matmuls(ps, ot, nt0, nchunks, phase):
                  for qq in range(nchunks * 2):
                      n0 = nt0 * 512 + qq * 256
                      for p in phase:
                          nc.tensor.matmul(
                              ps[:, qq * 256:(qq + 1) * 256],
                              mtp[:, 2 * p:2 * p + 2,
                                  ot * 128:(ot + 1) * 128],
                              knp[:, 2 * p:2 * p + 2, n0:n0 + 256],
                              start=(p == 0), stop=(p == 1),
                              perf_mode=DR)

              for ti, (ot, nt0, nchunks) in enumerate(dtiles):
                  bias_ap = biascol_sb[:, ot:ot + 1]
                  wcols = nchunks * 512
                  ps = psD.tile([128, wcols], f32,
                                name=f"o{R}_{ti}", tag="ops")
                  d_matmuls(ps, ot, nt0, nchunks, (0, 1))
                  osb = opool.tile([128, wcols], bf16,
                                   name=f"os{R}_{ti}", tag="osb")
                  # GPSIMD cannot read PSUM, so alternate Act/DVE
                  if ti % 2 == 0:
                      nc.scalar.activation(osb[:, :], ps[:, :],
                                           AF.Identity, bias=bias_ap,
                                           scale=1.0 / MSCALE)
                  else:
                      nc.vector.tensor_scalar(osb[:, :], ps[:, :],
                                              1.0 / MSCALE, bias_ap,
                                              ALU.mult, ALU.add)
                  nc.sync.dma_start(
                      out[ot * 128:(ot + 1) * 128,
                          nt0 * 512:nt0 * 512 + wcols],
                      osb[:, :])
              psD_cm.__exit__(None, None, None)
    nc.compile()
    return nc


def _make_in_maps(in1, in2, temperature, proj_w, proj_b):
    import ml_dtypes
    bf16 = ml_dtypes.bfloat16
    fp8 = ml_dtypes.float8_e4m3
    in1 = np.ascontiguousarray(in1, dtype=np.float32).reshape(B, C, N)
    in2 = np.ascontiguousarray(in2, dtype=np.float32).reshape(B, C, N)
    temperature = np.asarray(temperature, dtype=np.float32).reshape(HEADS)
    proj_w = np.asarray(proj_w, dtype=np.float32)
    proj_b = np.asarray(proj_b, dtype=np.float32)

    # host-side input statistics (<1% of total FLOPs): L2 norms + scales
    EPS = 1e-12
    qn = np.maximum(np.sqrt((in1.astype(np.float64) ** 2).sum(-1)), EPS)  # [B, C]
    kn_ = np.maximum(np.sqrt((in2.astype(np.float64) ** 2).sum(-1)), EPS)
    s_q = (1.0 / qn).astype(np.float32)
    s_k = (1.0 / kn_).astype(np.float32)
    temp_c = temperature[np.arange(C) // HD]                              # [C]
    qh = (in1 * (128.0 * s_q)[:, :, None]).astype(np.float32)  # 128*q-hat
    kh = (in2 * (128.0 * s_k)[:, :, None]).astype(np.float32)  # 128*k-hat

    pwt = np.ascontiguousarray(proj_w.T).astype(bf16)
    biascol = np.ascontiguousarray(
        proj_b.reshape(CT, 128).T.astype(np.float32))                     # [128,CT]
    maskr = np.empty((128, WTOT), np.float32)
    for ct in range(CT):
        rows = (np.arange(ct * 128, (ct + 1) * 128) // HD)[:, None]
        cols = (np.arange(R0[ct], R0[ct] + WR[ct]) // HD)[None, :]
        maskr[:, OFF[ct]:OFF[ct] + WR[ct]] = np.where(rows == cols, 0.0, NEG)
    maskr = maskr.astype(bf16)

    # full-N q/k transpose, host-swizzled to contiguous per-partition
    # blocks of 8 n-tiles; identical for the two cores of a batch pair up
    # to group order (each core gets its OWN spatial half as groups 8..15,
    # which the kernel PE-transposes into the phase-D k operand)
    qk_by_batch = []
    for b in range(B):
        qk = np.concatenate([qh[b].T, kh[b].T], axis=-1)         # [N, 2C]
        qk = qk.reshape(NGR, 8, 128, 2 * C).transpose(0, 2, 1, 3)
        qk_by_batch.append(np.ascontiguousarray(qk).astype(fp8))
    identm = np.eye(128, dtype=np.float32).astype(fp8)

    in_maps = []
    for core in range(NCORES):
        b, h = core // NHALF, core % NHALF
        expsc = np.empty((128, CT), np.float32)
        skc = np.full((128, CT), MSCALE / 128.0, np.float32)
        for ct in range(CT):
            rows = np.arange(ct * 128, (ct + 1) * 128)
            expsc[:, ct] = temp_c[rows] / 16384.0
        cb16 = np.empty((128, CT * C + WTOT), bf16)
        for ct in range(CT):
            cb16[:, ct * C:(ct + 1) * C] = pwt[ct * 128:(ct + 1) * 128, :]
        cb16[:, CT * C:CT * C + WTOT] = maskr
        cf32 = np.concatenate([expsc, skc, biascol], axis=1).astype(np.float32)
        # own spatial half at positions 1..8: feeds the phase-D transposes
        qk = qk_by_batch[b]
        other = np.arange(8 - 8 * h, 16 - 8 * h)
        own = np.arange(8 * h, 8 * h + 8)
        order = np.r_[other[:1], own, other[1:]]
        in_maps.append({
            "qkt": np.ascontiguousarray(qk[order]),
            "cb16": np.ascontiguousarray(cb16),
            "cf32": np.ascontiguousarray(cf32),
            "ident": identm,
        })
    return in_maps


_NC_CACHE = {}


def _get_nc(nrep=1):
    if nrep not in _NC_CACHE:
        _NC_CACHE[nrep] = build_nc(nrep)
    return _NC_CACHE[nrep]


def run_cores(in_maps, trace=False):
    from concourse.bass_utils import run_bass_kernel_spmd
    nc = _get_nc()
    res = run_bass_kernel_spmd(nc, in_maps, core_ids=list(range(NCORES)),
                               trace=trace)
    return res


def kernel(in1, in2, temperature, proj_w, proj_b):
    in_maps = _make_in_maps(in1, in2, temperature, proj_w, proj_b)
    res = run_cores(in_maps, trace=False)
    full = np.empty((B, C, N), dtype=np.float32)
    for core in range(NCORES):
        b, h = core // NHALF, core % NHALF
        full[b, :, h * NL:(h + 1) * NL] = np.asarray(
            res.results[core]["out"], dtype=np.float32)
    return full.reshape(B, C, H, W)



# revision 31
# speedup vs baseline: 1.0025x; 1.0025x over previous
"""Trainium2 distributed kernel for channel-attention (XCA-style) module.

Reference computation (B=4, C=384, HEADS=8, HD=48, H=W=128, N=HW=16384):
  q = l2norm(in1.view(B,HEADS,HD,N), dim=-1)
  k = l2norm(in2.view(B,HEADS,HD,N), dim=-1)
  attn = softmax(q @ k^T * temperature, dim=-1)          # [B,HEADS,HD,HD]
  out  = attn @ k                                        # [B,HEADS,HD,N]
  out  = proj_w @ out + proj_b                           # 1x1 conv

Distribution: 2D over (batch, spatial-half) — core 2b+h owns batch b and
writes output spatial positions [h*8192, (h+1)*8192). The Gram matrix
q@k^T (contraction over ALL of N) is computed REDUNDANTLY on both cores of
a batch pair from the full fp8 q/k transpose — 6.3 MB of extra DMA per
core buys the removal of the inter-core AllReduce and its latency chain
entirely; there are NO collectives. Softmax + projection-fold run once per
core, the big output matmul covers only the core's spatial half, and the
host concatenates the halves.

Key algebraic tricks:
- Per-head attention + the 1x1-conv projection fuse into ONE matmul:
    final = (proj_w @ blockdiag(attn_h * s_k)) @ k,  s_k[d] = 1/||k_d||.
- The Gram is only needed on the 8 diagonal 48x48 head blocks. Each 128-row
  tile ct only needs head-aligned columns [r0_ct, r0_ct+w_ct) (144/192/144
  of 384), cutting Gram matmul cycles 2.4x. Softmax reads the f32 Gram
  straight out of PSUM; the -1e30 additive mask zeroes in-range off-block
  entries, and the result lands in a persistent pre-zeroed [128, C]
  block-diagonal tile so the fold matmul stays dense.
- q/k row norms and temperature are input statistics; the host precomputes
  the rank-1 logit scale s_q(c)*temp(h)*s_k(d) and the s_k output fold.
- proj_b is applied as a per-partition bias in the PSUM->SBUF output copy
  (scalar/vector engines alternating), not as an extra matmul.
- qkt is host-swizzled so every DMA is a single contiguous 6 KB/partition
  block; phase-D PSUM tiles span two banks so one wide copy retires two
  matmul accumulation groups.
Matmul operands are bf16/fp8 (fp32 accumulation in PSUM); softmax stays f32;
the output is written bf16 and upcast to f32 on the host.
"""

import sys

import numpy as np

try:
    import concourse  # noqa: F401
except ImportError:
    sys.path.insert(0, "/opt/trn_rl_repo")

B, C, HEADS, HD = 4, 384, 8, 48
H = W = 128
N = H * W            # 16384
NCORES = 8
NHALF = 2            # spatial halves per batch
NL = N // NHALF      # 8192 output spatial positions per core
NTF = N // 128       # 128 n-tiles for the (full-N) Gram
NGR = NTF // 8       # 16 swizzled qkt groups
CT = C // 128        # 3 channel tiles
NT4 = NL // 512      # 16 output n-chunks
G4 = 4               # output n-chunks staged per SBUF tile
NEG = -1.0e30
R0 = [0, 96, 240]    # first needed Gram column per channel tile
WR = [144, 192, 144]  # needed Gram column count per channel tile
OFF = [0, 144, 336]  # offsets of the restricted tiles in packed buffers
WTOT = 480
MSCALE = 4096.0      # fp8 range lift for the folded projection matrix


def build_nc(nrep=1):
    import concourse.bass as bass
    import concourse.bacc as bacc
    import concourse.mybir as mybir
    from concourse.tile import TileContext

    f32 = mybir.dt.float32
    bf16 = mybir.dt.bfloat16
    fp8 = mybir.dt.float8e4
    AX = mybir.AxisListType
    AF = mybir.ActivationFunctionType
    DR = mybir.MatmulPerfMode.DoubleRow
    ALU = mybir.AluOpType

    nc = bacc.Bacc()
    nc._allow_low_precision_reason = "bf16/fp8 matmul operands are intentional"

    qkt = nc.declare_dram_parameter("qkt", [NGR, 128, 8, 2 * C], fp8,
                                    isOutput=False)
    # packed constants: one bf16 block (pwt | maskr) and one f32 block
    # (exp-scale | skc | biascol) -> few DMAs; identity for PE transposes
    cb16 = nc.declare_dram_parameter("cb16", [128, CT * C + WTOT], bf16,
                                     isOutput=False)
    cf32 = nc.declare_dram_parameter("cf32", [128, 3 * CT], f32,
                                     isOutput=False)
    ident = nc.declare_dram_parameter("ident", [128, 128], fp8,
                                      isOutput=False)
    out = nc.declare_dram_parameter("out", [C, NL], bf16, isOutput=True)

    with TileContext(nc) as tc:
        with (
            tc.tile_pool(name="const", bufs=1) as cpool,
            tc.tile_pool(name="qk", bufs=4) as qkpool,
            tc.tile_pool(name="work", bufs=8) as wpool,
            tc.tile_pool(name="osb", bufs=6) as opool,
        ):
            # ---- constants (two packed DMAs, issued after the first
            # qkt load of rep 0 so the Gram's data goes out first) ----
            cb16_sb = cpool.tile([128, CT * C + WTOT], bf16)
            cf32_sb = cpool.tile([128, 3 * CT], f32)
            pwt_sb = [cb16_sb[:, ct * C:(ct + 1) * C] for ct in range(CT)]
            maskr_sb = cb16_sb[:, CT * C:CT * C + WTOT]
            expsc_sb = cf32_sb[:, 0:CT]
            skc_sb = cf32_sb[:, CT:2 * CT]
            biascol_sb = cf32_sb[:, 2 * CT:3 * CT]
            # persistent block-diagonal attention tiles; zeros off the
            # restricted ranges are never rewritten
            bd_sb = []
            for ct in range(CT):
                t = cpool.tile([128, C], bf16, name=f"bd{ct}")
                nc.vector.memset(t[:, :], 0.0)
                bd_sb.append(t)
            # persistent fp8 operand tiles for the DoubleRow phase D: dim1
            # indexes the contraction k-tile; slot 3 is a zero pad so the
            # 384-deep contraction runs as two 256-deep DoubleRow pairs
            knp = cpool.tile([128, 4, NL], fp8, name="knp")
            mtp = cpool.tile([128, 4, C], fp8, name="mtp")
            ident_sb = cpool.tile([128, 128], fp8, name="ident")
            nc.gpsimd.memset(knp[:, 3, :], 0.0)
            nc.gpsimd.memset(mtp[:, 3, :], 0.0)

            for rep in range(nrep):
              R = str(rep)

              # ---- phase A: full-N Gram (redundant per batch pair),
              # fp8 DoubleRow: each matmul contracts TWO 128-spatial tiles.
              # The host orders each core's OWN spatial half into groups
              # 8..15; those k-columns are PE-transposed into knp, replacing
              # a separate kn DMA stream entirely ----
              psA_cm = tc.tile_pool(name=f"psA{R}", bufs=1, space="PSUM")
              psA = psA_cm.__enter__()
              psT_cm = tc.tile_pool(name=f"psT{R}", bufs=2, space="PSUM")
              psT = psT_cm.__enter__()
              gram_ps = [psA.tile([128, WR[ct]], f32, name=f"g{R}_{ct}",
                                  tag=f"gram{ct}")
                         for ct in range(CT)]
              ncopy = 0
              for g8 in range(NGR):
                  qk8 = qkpool.tile([128, 8, 2 * C], fp8, name=f"qk{R}_{g8}",
                                    tag="qk", bufs=6)
                  nc.sync.dma_start(qk8[:, :, :], qkt[g8, :, :, :])
                  if rep == 0 and g8 == 0:
                      nc.sync.dma_start(cf32_sb[:, :], cf32[:, :])
                      nc.sync.dma_start(ident_sb[:, :], ident[:, :])
                  if rep == 0 and g8 == 12:
                      # maskr: needed right at softmax; pwt follows the
                      # last qkt group (needed ~2us later at the fold)
                      nc.sync.dma_start(cb16_sb[:, CT * C:],
                                        cb16[:, CT * C:])
                  gm = range(4) if g8 < NGR - 1 else None
                  if gm is not None:
                      for p in gm:
                          for ct in range(CT):
                              nc.tensor.matmul(
                                  gram_ps[ct][:, :],
                                  qk8[:, 2 * p:2 * p + 2,
                                      ct * 128:(ct + 1) * 128],
                                  qk8[:, 2 * p:2 * p + 2,
                                      C + R0[ct]:C + R0[ct] + WR[ct]],
                                  start=(g8 == 0 and p == 0), stop=False,
                                  perf_mode=DR,
                              )
                  else:
                      # final group ct-major: tile ct finishes (stop=True) as
                      # early as possible so its softmax overlaps the rest
                      for ct in range(CT):
                          for p in range(4):
                              nc.tensor.matmul(
                                  gram_ps[ct][:, :],
                                  qk8[:, 2 * p:2 * p + 2,
                                      ct * 128:(ct + 1) * 128],
                                  qk8[:, 2 * p:2 * p + 2,
                                      C + R0[ct]:C + R0[ct] + WR[ct]],
                                  start=False, stop=(p == 3),
                                  perf_mode=DR,
                              )
                  if 1 <= g8 <= 8:
                      # own-half k columns -> knp via PE transpose; fp8
                      # transpose writes PSUM at element step 2, so the
                      # tile is 2x wide and read back strided. Early
                      # positions keep the Act/DVE copies clear of the
                      # softmax and the PE clear of the fold.
                      n0 = (g8 - 1) * 1024
                      for j in range(CT):
                          pt = psT.tile([128, 2 * 8 * 128], fp8,
                                        name=f"t{R}_{g8}_{j}", tag="tp")
                          for ht in range(8):
                              nc.tensor.transpose(
                                  pt[:, ht * 256:(ht + 1) * 256:2],
                                  qk8[:, ht, C + j * 128:C + (j + 1) * 128],
                                  ident_sb[:, :])
                          if ncopy % 2 == 0:
                              nc.scalar.activation(
                                  knp[:, j, n0:n0 + 1024],
                                  pt[:, 0:2048:2], AF.Copy)
                          else:
                              nc.vector.tensor_copy(
                                  knp[:, j, n0:n0 + 1024], pt[:, 0:2048:2])
                          ncopy += 1
              if rep == 0:
                  nc.sync.dma_start(cb16_sb[:, :CT * C], cb16[:, :CT * C])

              # ---- phase C: masked softmax straight from PSUM, fused M^T ----
              # Host ships q/128||q||, k/128||k|| in fp8 (power-of-2 scaling
              # is exact), so logits = gram * temp/16384, folded into the
              # exp's per-partition scale; the -1e30 mask survives the tiny
              # scale (exp(-6e25) == 0).
              for ct in range(CT):
                  w = WR[ct]
                  l = wpool.tile([128, w], f32, name=f"l{R}_{ct}", tag=f"l{ct}")
                  nc.vector.tensor_add(
                      l[:, :], gram_ps[ct][:, :],
                      maskr_sb[:, OFF[ct]:OFF[ct] + w])
                  e = wpool.tile([128, w], f32, name=f"e{R}_{ct}", tag=f"e{ct}")
                  ssum = wpool.tile([128, 1], f32, name=f"ss{R}_{ct}",
                                    tag=f"ss{ct}")
                  # row sum accumulated inside the exp pass (no DVE reduce)
                  nc.scalar.activation(e[:, :], l[:, :], AF.Exp,
                                       scale=expsc_sb[:, ct:ct + 1],
                                       accum_out=ssum[:, :])
                  nc.vector.reciprocal(ssum[:, :], ssum[:, :])
                  # normalized softmax written straight into the persistent
                  # block-diagonal tile (off-range stays zero)
                  nc.vector.tensor_scalar_mul(
                      bd_sb[ct][:, R0[ct]:R0[ct] + w], e[:, :], ssum[:, 0:1])
              psT_cm.__exit__(None, None, None)
              psA_cm.__exit__(None, None, None)

              # ---- fold + phase D share ONE PSUM pool (4 x 2-bank ring):
              # the three fold tiles are the first ring slots, so the first
              # true phase-D tile lands in virgin banks with no pool-release
              # cascade in between ----
              psD_cm = tc.tile_pool(name=f"psD{R}", bufs=4, space="PSUM")
              psD = psD_cm.__enter__()
              # fold matmuls ct-outer: each ct's contribution to all three
              # M^T blocks issues as soon as that ct's softmax lands, so the
              # fold overlaps the remaining softmax columns
              mt_ps = [psD.tile([128, 2 * 512], f32, name=f"mt{R}_{j}",
                                tag="ops") for j in range(CT)]
              for ct in range(CT):
                  for j in range(CT):
                      nc.tensor.matmul(
                          mt_ps[j][:, 0:C],
                          bd_sb[ct][:, j * 128:(j + 1) * 128],
                          pwt_sb[ct],
                          start=(ct == 0), stop=(ct == CT - 1))
              # fold MSCALE/128 (the kh stream carries 128*k-hat) into the
              # PSUM->SBUF fp8 quantization copies, balanced across Act/DVE
              # so the last mtp block lands as early as possible
              nc.scalar.activation(mtp[:, 0, :], mt_ps[0][:, 0:C],
                                   AF.Copy, scale=skc_sb[:, 0:1])
              nc.vector.tensor_scalar_mul(
                  mtp[:, 1, :], mt_ps[1][:, 0:C], skc_sb[:, 1:2])
              half = C // 2
              nc.scalar.activation(mtp[:, 2, 0:half], mt_ps[2][:, 0:half],
                                   AF.Copy, scale=skc_sb[:, 2:3])
              nc.vector.tensor_scalar_mul(
                  mtp[:, 2, half:C], mt_ps[2][:, half:C], skc_sb[:, 2:3])
              dtiles = []
              for q2 in range(8):
                  for ot in range(CT):
                      if q2 == 7 and ot == 2:
                          dtiles.append((ot, 14, 1))
                          dtiles.append((ot, 15, 1))
                      else:
                          dtiles.append((ot, q2 * 2, 2))
              def d_matmuls(ps, ot, nt0, nchunks, phase):
                  for qq in range(nchunks * 2):
                      n0 = nt0 * 512 + qq * 256
                      for p in phase:
                          nc.tensor.matmul(
                              ps[:, qq * 256:(qq + 1) * 256],
                              mtp[:, 2 * p:2 * p + 2,
                                  ot * 128:(ot + 1) * 128],
                              knp[:, 2 * p:2 * p + 2, n0:n0 + 256],
                              start=(p == 0), stop=(p == 1),
                              perf_mode=DR)

              for ti, (ot, nt0, nchunks) in enumerate(dtiles):
                  bias_ap = biascol_sb[:, ot:ot + 1]
                  wcols = nchunks * 512
                  ps = psD.tile([128, wcols], f32,
                                name=f"o{R}_{ti}", tag="ops")
                  d_matmuls(ps, ot, nt0, nchunks, (0, 1))
                  osb = opool.tile([128, wcols], bf16,
                                   name=f"os{R}_{ti}", tag="osb")
                  # GPSIMD cannot read PSUM, so alternate Act/DVE
                  if ti % 2 == 0:
                      nc.scalar.activation(osb[:, :], ps[:, :],
                                           AF.Identity, bias=bias_ap,
                                           scale=1.0 / MSCALE)
                  else:
                      nc.vector.tensor_scalar(osb[:, :], ps[:, :],
                                              1.0 / MSCALE, bias_ap,
                                              ALU.mult, ALU.add)
                  nc.sync.dma_start(
                      out[ot * 128:(ot + 1) * 128,
                          nt0 * 512:nt0 * 512 + wcols],
                      osb[:, :])
              psD_cm.__exit__(None, None, None)
    nc.compile()
    return nc


def _make_in_maps(in1, in2, temperature, proj_w, proj_b):
    import ml_dtypes
    bf16 = ml_dtypes.bfloat16
    fp8 = ml_dtypes.float8_e4m3
    in1 = np.ascontiguousarray(in1, dtype=np.float32).reshape(B, C, N)
    in2 = np.ascontiguousarray(in2, dtype=np.float32).reshape(B, C, N)
    temperature = np.asarray(temperature, dtype=np.float32).reshape(HEADS)
    proj_w = np.asarray(proj_w, dtype=np.float32)
    proj_b = np.asarray(proj_b, dtype=np.float32)

    # host-side input statistics (<1% of total FLOPs): L2 norms + scales
    EPS = 1e-12
    qn = np.maximum(np.sqrt((in1.astype(np.float64) ** 2).sum(-1)), EPS)  # [B, C]
    kn_ = np.maximum(np.sqrt((in2.astype(np.float64) ** 2).sum(-1)), EPS)
    s_q = (1.0 / qn).astype(np.float32)
    s_k = (1.0 / kn_).astype(np.float32)
    temp_c = temperature[np.arange(C) // HD]                              # [C]
    qh = (in1 * (128.0 * s_q)[:, :, None]).astype(np.float32)  # 128*q-hat
    kh = (in2 * (128.0 * s_k)[:, :, None]).astype(np.float32)  # 128*k-hat

    pwt = np.ascontiguousarray(proj_w.T).astype(bf16)
    biascol = np.ascontiguousarray(
        proj_b.reshape(CT, 128).T.astype(np.float32))                     # [128,CT]
    maskr = np.empty((128, WTOT), np.float32)
    for ct in range(CT):
        rows = (np.arange(ct * 128, (ct + 1) * 128) // HD)[:, None]
        cols = (np.arange(R0[ct], R0[ct] + WR[ct]) // HD)[None, :]
        maskr[:, OFF[ct]:OFF[ct] + WR[ct]] = np.where(rows == cols, 0.0, NEG)
    maskr = maskr.astype(bf16)

    # full-N q/k transpose, host-swizzled to contiguous per-partition
    # blocks of 8 n-tiles; identical for the two cores of a batch pair up
    # to group order (each core gets its OWN spatial half as groups 8..15,
    # which the kernel PE-transposes into the phase-D k operand)
    qk_by_batch = []
    for b in range(B):
        qk = np.concatenate([qh[b].T, kh[b].T], axis=-1)         # [N, 2C]
        qk = qk.reshape(NGR, 8, 128, 2 * C).transpose(0, 2, 1, 3)
        qk_by_batch.append(np.ascontiguousarray(qk).astype(fp8))
    identm = np.eye(128, dtype=np.float32).astype(fp8)

    in_maps = []
    for core in range(NCORES):
        b, h = core // NHALF, core % NHALF
        expsc = np.empty((128, CT), np.float32)
        skc = np.full((128, CT), MSCALE / 128.0, np.float32)
        for ct in range(CT):
            rows = np.arange(ct * 128, (ct + 1) * 128)
            expsc[:, ct] = temp_c[rows] / 16384.0
        cb16 = np.empty((128, CT * C + WTOT), bf16)
        for ct in range(CT):
            cb16[:, ct * C:(ct + 1) * C] = pwt[ct * 128:(ct + 1) * 128, :]
        cb16[:, CT * C:CT * C + WTOT] = maskr
        cf32 = np.concatenate([expsc, skc, biascol], axis=1).astype(np.float32)
        # own spatial half at positions 1..8: feeds the phase-D transposes
        qk = qk_by_batch[b]
        other = np.arange(8 - 8 * h, 16 - 8 * h)
        own = np.arange(8 * h, 8 * h + 8)
        order = np.r_[other[:1], own, other[1:]]
        in_maps.append({
            "qkt": np.ascontiguousarray(qk[order]),
            "cb16": np.ascontiguousarray(cb16),
            "cf32": np.ascontiguousarray(cf32),
            "ident": identm,
        })
    return in_maps


_NC_CACHE = {}


def _get_nc(nrep=1):
    if nrep not in _NC_CACHE:
        _NC_CACHE[nrep] = build_nc(nrep)
    return _NC_CACHE[nrep]


def run_cores(in_maps, trace=False):
    from concourse.bass_utils import run_bass_kernel_spmd
    nc = _get_nc()
    res = run_bass_kernel_spmd(nc, in_maps, core_ids=list(range(NCORES)),
                               trace=trace)
    return res


def kernel(in1, in2, temperature, proj_w, proj_b):
    in_maps = _make_in_maps(in1, in2, temperature, proj_w, proj_b)
    res = run_cores(in_maps, trace=False)
    full = np.empty((B, C, N), dtype=np.float32)
    for core in range(NCORES):
        b, h = core // NHALF, core % NHALF
        full[b, :, h * NL:(h + 1) * NL] = np.asarray(
            res.results[core]["out"], dtype=np.float32)
    return full.reshape(B, C, H, W)

